# revision 19
# baseline (speedup 1.0000x reference)
"""Trainium2 Bass kernel for nn_Dense_56779467653682.

Computes out = scale * x @ (2*kernel - 1) where x:[8,2048,4096] f32,
kernel:[4096,4096] bool, scale scalar f32 (= 1/64).

Strategy: data-parallel over the 16384 tokens across 8 NeuronCores
(2048 tokens/core). The ternary weight (+-scale, exact in bf16 since
scale is a power of two) is folded on the host into a bf16 weight
matrix, and x is cast to bf16 and pre-transposed/tiled on the host so
the device kernel is a pure dense matmul:

    per core: out[2048, 4096] f32 = x_bf16[2048, 4096] @ w_bf16[4096, 4096]

Device tiling (per core):
  - contraction K=4096 -> 32 k-tiles of 128 (partition dim)
  - tokens M=2048 -> 16 m-tiles of 128 (PSUM partition dim, lhsT free dim)
  - features N=4096 -> 8 n-chunks of 512 (PSUM free dim = one bank)
  All 16 xT m-tiles stay resident in SBUF (128 KB/partition); w streams
  once in 4 MB n-chunks (double buffered); each output tile accumulates
  32 back-to-back matmuls in one PSUM bank, is copied to SBUF on the
  DVE, and DMA'd out.
"""

import numpy as np
import ml_dtypes

BATCH, SEQ, IN_DIM, FEATURES = 8, 2048, 4096, 4096
N_CORES = 8
TOKENS = BATCH * SEQ
TOK_PER_CORE = TOKENS // N_CORES  # 2048
P = 128                           # partitions / tile edge
KT = IN_DIM // P                  # 32 k-tiles
MT = TOK_PER_CORE // P            # 16 m-tiles
NF = 512                          # features per n-chunk (one PSUM bank of f32)
NT = FEATURES // NF               # 8 n-chunks

_BF16 = ml_dtypes.bfloat16

_cache = {}


def _build_program():
    """Build + compile the per-core Bass/Tile program (SPMD, same on all cores)."""
    import concourse.bacc as bacc
    import concourse.mybir as mybir
    from concourse.tile import TileContext

    nc = bacc.Bacc("TRN2", target_bir_lowering=False, debug=False)

    xs_d = nc.dram_tensor("xs", [MT, P, KT, P], mybir.dt.bfloat16, kind="ExternalInput")
    ws_d = nc.dram_tensor("ws", [NT, P, KT, NF], mybir.dt.bfloat16, kind="ExternalInput")
    out_d = nc.dram_tensor("out", [TOK_PER_CORE, FEATURES], mybir.dt.float32, kind="ExternalOutput")

    KG = 4                 # k-tiles per w sub-tile (fine-grained RAW deps)
    NSUB = KT // KG        # 8 sub-tiles per n-chunk
    WARMUP_MMS = 30        # dummy matmuls to lift HAM to K=8/8 during input DMA

    with TileContext(nc) as tc:
        with (
            tc.tile_pool(name="xpool", bufs=1) as xpool,
            tc.tile_pool(name="wpool", bufs=2 * NSUB) as wpool,
            tc.tile_pool(name="epool", bufs=4) as epool,
            tc.tile_pool(name="warm", bufs=1) as warm,
            tc.tile_pool(name="psum", bufs=4, space="PSUM") as pp,
            tc.tile_pool(name="psumw", bufs=1, space="PSUM") as ppw,
        ):
            # PE warmup: the HAM clock gate only reaches 2.4 GHz after ~3.4us
            # of sustained PE activity. Burn the initial DMA wait on dummy
            # matmuls so the real ones start at full clock.
            wu = warm.tile([P, 256], mybir.dt.bfloat16, name="wu")
            nc.gpsimd.memset(wu[:], 0.0)
            wups = ppw.tile([P, 256], mybir.dt.float32, name="wups")
            for _ in range(WARMUP_MMS):
                nc.tensor.matmul(wups[:], wu[:, :P], wu[:], start=True, stop=True)

            # Resident xT tiles: [k-partition, k-tile, token] per m-tile.
            # w streams as [128, KG, 512] sub-tiles (512 KB) so matmuls wait
            # on small DMAs; 16 pool slots hold the live chunk plus a fully
            # prefetched next chunk. x loads go through the scalar engine's
            # HWDGE so their ~0.6us/DMA descriptor issue runs in parallel
            # with the w stream on the sync engine.
            w_tiles = [None] * NT

            def w_sub(nt, g):
                wt = wpool.tile(
                    [P, KG, NF], mybir.dt.bfloat16, name=f"w_{nt}_{g}", tag="w"
                )
                nc.sync.dma_start(out=wt[:], in_=ws_d[nt, :, g * KG:(g + 1) * KG, :])
                return wt

            def load_w(nt):
                w_tiles[nt] = [w_sub(nt, g) for g in range(NSUB)]

            def x_tile(mt):
                xt = xpool.tile([P, KT, P], mybir.dt.bfloat16, name=f"xs_t{mt}")
                nc.scalar.dma_start(out=xt[:], in_=xs_d[mt])
                return xt

            # Ramp: the first two m-tiles are loaded in k-halves (512 KB) so
            # the first matmul waits on ~1 MB instead of ~2 MB.
            KH = KT // 2
            xs_sub = {0: [], 1: []}
            for h in range(2):
                for mt in (0, 1):
                    xh = xpool.tile([P, KH, P], mybir.dt.bfloat16, name=f"xs_t{mt}_{h}")
                    nc.scalar.dma_start(
                        out=xh[:], in_=xs_d[mt, :, h * KH:(h + 1) * KH, :]
                    )
                    xs_sub[mt].append(xh)

            load_w(0)

            xs_t = [None, None]
            for mt in range(2, MT):
                xs_t.append(x_tile(mt))

            def x_slice(mt, ko):
                if mt < 2:
                    return xs_sub[mt][ko // KH][:, ko % KH, :]
                return xs_t[mt][:, ko, :]

            def w_slice(nt, ko):
                return w_tiles[nt][ko // KG][:, ko % KG, :]

            def finish_tile(nt, mt, ps):
                ev = epool.tile([P, NF], mybir.dt.float32, name="ev", tag="ev")
                nc.vector.tensor_copy(ev[:], ps[:])
                nc.sync.dma_start(
                    out=out_d[mt * P:(mt + 1) * P, nt * NF:(nt + 1) * NF],
                    in_=ev[:],
                )

            for nt in range(NT):
                if w_tiles[nt] is None:
                    load_w(nt)
                if nt == 0:
                    # Ramp: the first w chunk is still streaming in, and the
                    # PE eats one (m-tile, w-sub) block faster than its DMA.
                    # Interleave m-tile pairs (two open PSUM groups) so each
                    # w sub-tile feeds 2x the PE work and the DMA keeps up
                    # from the very first matmul.
                    for mp in range(0, 4, 2):
                        ps_a = pp.tile([P, NF], mybir.dt.float32, name="ps", tag="ps")
                        ps_b = pp.tile([P, NF], mybir.dt.float32, name="ps2", tag="ps")
                        for g in range(NSUB):
                            for mt, ps in ((mp, ps_a), (mp + 1, ps_b)):
                                for kk in range(KG):
                                    ko = g * KG + kk
                                    nc.tensor.matmul(
                                        ps[:],
                                        x_slice(mt, ko),
                                        w_slice(nt, ko),
                                        start=(ko == 0),
                                        stop=(ko == KT - 1),
                                    )
                        finish_tile(nt, mp, ps_a)
                        finish_tile(nt, mp + 1, ps_b)
                    mts = range(4, MT)
                else:
                    mts = range(MT)
                for mt in mts:
                    last = nt == NT - 1 and mt == MT - 1
                    if not last:
                        ps = pp.tile([P, NF], mybir.dt.float32, name="ps", tag="ps")
                        for ko in range(KT):
                            nc.tensor.matmul(
                                ps[:],
                                x_slice(mt, ko),
                                w_slice(nt, ko),
                                start=(ko == 0),
                                stop=(ko == KT - 1),
                            )
                        finish_tile(nt, mt, ps)
                    else:
                        # Final tile: two N=256 accumulation groups so the
                        # first half's evac + store overlap the second half's
                        # matmuls, shortening the kernel tail.
                        for half in range(2):
                            cs = slice(half * (NF // 2), (half + 1) * (NF // 2))
                            ps = pp.tile(
                                [P, NF // 2], mybir.dt.float32, name="psl", tag="ps"
                            )
                            for ko in range(KT):
                                nc.tensor.matmul(
                                    ps[:],
                                    x_slice(mt, ko),
                                    w_slice(nt, ko)[:, cs],
                                    start=(ko == 0),
                                    stop=(ko == KT - 1),
                                )
                            ev = epool.tile(
                                [P, NF // 2], mybir.dt.float32, name="evl", tag="ev"
                            )
                            nc.vector.tensor_copy(ev[:], ps[:])
                            nc.sync.dma_start(
                                out=out_d[mt * P:(mt + 1) * P,
                                          nt * NF + half * (NF // 2):
                                          nt * NF + (half + 1) * (NF // 2)],
                                in_=ev[:],
                            )

    nc.compile()
    return nc


def _prep_inputs(x, kern, scale):
    """Host-side: fold scale into ternary bf16 weights; cast+tile x per core."""
    s = float(np.asarray(scale))
    # w[k, f] = +-scale, exact in bf16 when scale is a power of two.
    w = np.where(np.asarray(kern), np.float32(s), np.float32(-s)).astype(_BF16)
    # ws[nt, kp, ko, n] = w[ko*128 + kp, nt*512 + n]
    ws = np.ascontiguousarray(
        w.reshape(KT, P, NT, NF).transpose(2, 1, 0, 3)
    )

    xf = np.asarray(x).reshape(TOKENS, IN_DIM).astype(_BF16)
    in_maps = []
    for c in range(N_CORES):
        xc = xf[c * TOK_PER_CORE:(c + 1) * TOK_PER_CORE]
        # xs[mt, kp, ko, mi] = xc[mt*128 + mi, ko*128 + kp]
        xs = np.ascontiguousarray(
            xc.reshape(MT, P, KT, P).transpose(0, 3, 2, 1)
        )
        in_maps.append({"xs": xs, "ws": ws})
    return in_maps


def _run(inputs, trace=False, tmpdir=None):
    from concourse.bass_utils import run_bass_kernel_spmd

    if "nc" not in _cache:
        _cache["nc"] = _build_program()
    nc = _cache["nc"]

    in_maps = _prep_inputs(inputs["x"], inputs["kernel"], inputs["scale"])
    res = run_bass_kernel_spmd(
        nc, in_maps, core_ids=list(range(N_CORES)), trace=trace, tmpdir=tmpdir
    )
    out = np.concatenate(
        [res.results[c]["out"][None] for c in range(N_CORES)], axis=0
    ).reshape(BATCH, SEQ, FEATURES)
    return np.ascontiguousarray(out.astype(np.float32, copy=False)), res


def kernel(**inputs):
    out, _ = _run(inputs, trace=False)
    return out


# revision 21
# speedup vs baseline: 1.0072x; 1.0072x over previous
"""Trainium2 Bass kernel for nn_Dense_56779467653682.

Computes out = scale * x @ (2*kernel - 1) where x:[8,2048,4096] f32,
kernel:[4096,4096] bool, scale scalar f32 (= 1/64).

Strategy: data-parallel over the 16384 tokens across 8 NeuronCores
(2048 tokens/core). The ternary weight (+-scale, exact in bf16 since
scale is a power of two) is folded on the host into a bf16 weight
matrix, and x is cast to bf16 and pre-transposed/tiled on the host so
the device kernel is a pure dense matmul:

    per core: out[2048, 4096] f32 = x_bf16[2048, 4096] @ w_bf16[4096, 4096]

Device tiling (per core):
  - contraction K=4096 -> 32 k-tiles of 128 (partition dim)
  - tokens M=2048 -> 16 m-tiles of 128 (PSUM partition dim, lhsT free dim)
  - features N=4096 -> 8 n-chunks of 512 (PSUM free dim = one bank)
  All 16 xT m-tiles stay resident in SBUF (128 KB/partition); w streams
  once in 4 MB n-chunks (double buffered); each output tile accumulates
  32 back-to-back matmuls in one PSUM bank, is copied to SBUF on the
  DVE, and DMA'd out.
"""

import numpy as np
import ml_dtypes

BATCH, SEQ, IN_DIM, FEATURES = 8, 2048, 4096, 4096
N_CORES = 8
TOKENS = BATCH * SEQ
TOK_PER_CORE = TOKENS // N_CORES  # 2048
P = 128                           # partitions / tile edge
KT = IN_DIM // P                  # 32 k-tiles
MT = TOK_PER_CORE // P            # 16 m-tiles
NF = 512                          # features per n-chunk (one PSUM bank of f32)
NT = FEATURES // NF               # 8 n-chunks

_BF16 = ml_dtypes.bfloat16

_cache = {}


def _build_program():
    """Build + compile the per-core Bass/Tile program (SPMD, same on all cores)."""
    import concourse.bacc as bacc
    import concourse.mybir as mybir
    from concourse.tile import TileContext

    nc = bacc.Bacc("TRN2", target_bir_lowering=False, debug=False)

    xs_d = nc.dram_tensor("xs", [MT, P, KT, P], mybir.dt.bfloat16, kind="ExternalInput")
    ws_d = nc.dram_tensor("ws", [NT, P, KT, NF], mybir.dt.bfloat16, kind="ExternalInput")
    out_d = nc.dram_tensor("out", [TOK_PER_CORE, FEATURES], mybir.dt.float32, kind="ExternalOutput")

    KG = 4                 # k-tiles per w sub-tile (fine-grained RAW deps)
    NSUB = KT // KG        # 8 sub-tiles per n-chunk
    WARMUP_MMS = 30        # dummy matmuls to lift HAM to K=8/8 during input DMA

    with TileContext(nc) as tc:
        with (
            tc.tile_pool(name="xpool", bufs=1) as xpool,
            tc.tile_pool(name="wpool", bufs=2 * NSUB) as wpool,
            tc.tile_pool(name="epool", bufs=4) as epool,
            tc.tile_pool(name="warm", bufs=1) as warm,
            tc.tile_pool(name="psum", bufs=4, space="PSUM") as pp,
            tc.tile_pool(name="psumw", bufs=1, space="PSUM") as ppw,
        ):
            # PE warmup: the HAM clock gate only reaches 2.4 GHz after ~3.4us
            # of sustained PE activity. Burn the initial DMA wait on dummy
            # matmuls so the real ones start at full clock.
            wu = warm.tile([P, 256], mybir.dt.bfloat16, name="wu")
            nc.gpsimd.memset(wu[:], 0.0)
            wups = ppw.tile([P, 256], mybir.dt.float32, name="wups")
            for _ in range(WARMUP_MMS):
                nc.tensor.matmul(wups[:], wu[:, :P], wu[:], start=True, stop=True)

            # Resident xT tiles: [k-partition, k-tile, token] per m-tile.
            # w streams as [128, KG, 512] sub-tiles (512 KB) so matmuls wait
            # on small DMAs; 16 pool slots hold the live chunk plus a fully
            # prefetched next chunk. All loads share the sync engine's HWDGE
            # queue: the single FIFO keeps the ramp's arrival order exactly
            # the consumption order (a second engine's stream interleaves on
            # the DMA rings and delays the pieces the PE is waiting on).
            w_tiles = [None] * NT

            def w_sub(nt, g):
                wt = wpool.tile(
                    [P, KG, NF], mybir.dt.bfloat16, name=f"w_{nt}_{g}", tag="w"
                )
                nc.sync.dma_start(out=wt[:], in_=ws_d[nt, :, g * KG:(g + 1) * KG, :])
                return wt

            def load_w(nt):
                w_tiles[nt] = [w_sub(nt, g) for g in range(NSUB)]

            def x_tile(mt):
                xt = xpool.tile([P, KT, P], mybir.dt.bfloat16, name=f"xs_t{mt}")
                nc.sync.dma_start(out=xt[:], in_=xs_d[mt])
                return xt

            # Ramp: first two m-tiles in k-halves (512 KB), interleaved with
            # the first w chunk's pieces in exactly the order the
            # pair-interleaved matmuls below consume them.
            KH = KT // 2
            xs_sub = {0: [], 1: []}

            def x_half(mt, h):
                xh = xpool.tile([P, KH, P], mybir.dt.bfloat16, name=f"xs_t{mt}_{h}")
                nc.sync.dma_start(
                    out=xh[:], in_=xs_d[mt, :, h * KH:(h + 1) * KH, :]
                )
                xs_sub[mt].append(xh)

            x_half(0, 0)
            w0 = [w_sub(0, 0)]
            x_half(1, 0)
            w0 += [w_sub(0, g) for g in range(1, NSUB // 2)]
            x_half(0, 1)
            x_half(1, 1)
            w0 += [w_sub(0, g) for g in range(NSUB // 2, NSUB)]
            w_tiles[0] = w0

            xs_t = [None, None]
            for mt in range(2, MT):
                xs_t.append(x_tile(mt))

            def x_slice(mt, ko):
                if mt < 2:
                    return xs_sub[mt][ko // KH][:, ko % KH, :]
                return xs_t[mt][:, ko, :]

            def w_slice(nt, ko):
                return w_tiles[nt][ko // KG][:, ko % KG, :]

            def finish_tile(nt, mt, ps):
                ev = epool.tile([P, NF], mybir.dt.float32, name="ev", tag="ev")
                nc.vector.tensor_copy(ev[:], ps[:])
                nc.sync.dma_start(
                    out=out_d[mt * P:(mt + 1) * P, nt * NF:(nt + 1) * NF],
                    in_=ev[:],
                )

            for nt in range(NT):
                if w_tiles[nt] is None:
                    load_w(nt)
                if nt == 0:
                    # Ramp: the first w chunk is still streaming in, and the
                    # PE eats one (m-tile, w-sub) block faster than its DMA.
                    # Interleave m-tile pairs (two open PSUM groups) so each
                    # w sub-tile feeds 2x the PE work and the DMA keeps up
                    # from the very first matmul.
                    for mp in range(0, 4, 2):
                        ps_a = pp.tile([P, NF], mybir.dt.float32, name="ps", tag="ps")
                        ps_b = pp.tile([P, NF], mybir.dt.float32, name="ps2", tag="ps")
                        for g in range(NSUB):
                            for mt, ps in ((mp, ps_a), (mp + 1, ps_b)):
                                for kk in range(KG):
                                    ko = g * KG + kk
                                    nc.tensor.matmul(
                                        ps[:],
                                        x_slice(mt, ko),
                                        w_slice(nt, ko),
                                        start=(ko == 0),
                                        stop=(ko == KT - 1),
                                    )
                        finish_tile(nt, mp, ps_a)
                        finish_tile(nt, mp + 1, ps_b)
                    mts = range(4, MT)
                else:
                    mts = range(MT)
                for mt in mts:
                    ps = pp.tile([P, NF], mybir.dt.float32, name="ps", tag="ps")
                    for ko in range(KT):
                        nc.tensor.matmul(
                            ps[:],
                            x_slice(mt, ko),
                            w_slice(nt, ko),
                            start=(ko == 0),
                            stop=(ko == KT - 1),
                        )
                    finish_tile(nt, mt, ps)

    nc.compile()
    return nc


def _prep_inputs(x, kern, scale):
    """Host-side: fold scale into ternary bf16 weights; cast+tile x per core."""
    s = float(np.asarray(scale))
    # w[k, f] = +-scale, exact in bf16 when scale is a power of two.
    w = np.where(np.asarray(kern), np.float32(s), np.float32(-s)).astype(_BF16)
    # ws[nt, kp, ko, n] = w[ko*128 + kp, nt*512 + n]
    ws = np.ascontiguousarray(
        w.reshape(KT, P, NT, NF).transpose(2, 1, 0, 3)
    )

    xf = np.asarray(x).reshape(TOKENS, IN_DIM).astype(_BF16)
    in_maps = []
    for c in range(N_CORES):
        xc = xf[c * TOK_PER_CORE:(c + 1) * TOK_PER_CORE]
        # xs[mt, kp, ko, mi] = xc[mt*128 + mi, ko*128 + kp]
        xs = np.ascontiguousarray(
            xc.reshape(MT, P, KT, P).transpose(0, 3, 2, 1)
        )
        in_maps.append({"xs": xs, "ws": ws})
    return in_maps


def _run(inputs, trace=False, tmpdir=None):
    from concourse.bass_utils import run_bass_kernel_spmd

    if "nc" not in _cache:
        _cache["nc"] = _build_program()
    nc = _cache["nc"]

    in_maps = _prep_inputs(inputs["x"], inputs["kernel"], inputs["scale"])
    res = run_bass_kernel_spmd(
        nc, in_maps, core_ids=list(range(N_CORES)), trace=trace, tmpdir=tmpdir
    )
    out = np.concatenate(
        [res.results[c]["out"][None] for c in range(N_CORES)], axis=0
    ).reshape(BATCH, SEQ, FEATURES)
    return np.ascontiguousarray(out.astype(np.float32, copy=False)), res


def kernel(**inputs):
    out, _ = _run(inputs, trace=False)
    return out


# revision 23
# speedup vs baseline: 1.0083x; 1.0011x over previous
"""Trainium2 Bass kernel for nn_Dense_56779467653682.

Computes out = scale * x @ (2*kernel - 1) where x:[8,2048,4096] f32,
kernel:[4096,4096] bool, scale scalar f32 (= 1/64).

Strategy: data-parallel over the 16384 tokens across 8 NeuronCores
(2048 tokens/core). The ternary weight (+-scale, exact in bf16 since
scale is a power of two) is folded on the host into a bf16 weight
matrix, and x is cast to bf16 and pre-transposed/tiled on the host so
the device kernel is a pure dense matmul:

    per core: out[2048, 4096] f32 = x_bf16[2048, 4096] @ w_bf16[4096, 4096]

Device tiling (per core):
  - contraction K=4096 -> 32 k-tiles of 128 (partition dim)
  - tokens M=2048 -> 16 m-tiles of 128 (PSUM partition dim, lhsT free dim)
  - features N=4096 -> 8 n-chunks of 512 (PSUM free dim = one bank)
  All 16 xT m-tiles stay resident in SBUF (128 KB/partition); w streams
  once in 4 MB n-chunks (double buffered); each output tile accumulates
  32 back-to-back matmuls in one PSUM bank, is copied to SBUF on the
  DVE, and DMA'd out.
"""

import numpy as np
import ml_dtypes

BATCH, SEQ, IN_DIM, FEATURES = 8, 2048, 4096, 4096
N_CORES = 8
TOKENS = BATCH * SEQ
TOK_PER_CORE = TOKENS // N_CORES  # 2048
P = 128                           # partitions / tile edge
KT = IN_DIM // P                  # 32 k-tiles
MT = TOK_PER_CORE // P            # 16 m-tiles
NF = 512                          # features per n-chunk (one PSUM bank of f32)
NT = FEATURES // NF               # 8 n-chunks

_BF16 = ml_dtypes.bfloat16

_cache = {}


def _build_program():
    """Build + compile the per-core Bass/Tile program (SPMD, same on all cores)."""
    import concourse.bacc as bacc
    import concourse.mybir as mybir
    from concourse.tile import TileContext

    nc = bacc.Bacc("TRN2", target_bir_lowering=False, debug=False)

    xs_d = nc.dram_tensor("xs", [MT, P, KT, P], mybir.dt.bfloat16, kind="ExternalInput")
    ws_d = nc.dram_tensor("ws", [NT, P, KT, NF], mybir.dt.bfloat16, kind="ExternalInput")
    out_d = nc.dram_tensor("out", [TOK_PER_CORE, FEATURES], mybir.dt.float32, kind="ExternalOutput")

    KG = 4                 # k-tiles per w sub-tile (fine-grained RAW deps)
    NSUB = KT // KG        # 8 sub-tiles per n-chunk
    WARMUP_MMS = 26        # dummy matmuls to lift HAM to K=8/8 during input DMA

    with TileContext(nc) as tc:
        with (
            tc.tile_pool(name="xpool", bufs=1) as xpool,
            tc.tile_pool(name="wpool", bufs=2 * NSUB) as wpool,
            tc.tile_pool(name="epool", bufs=4) as epool,
            tc.tile_pool(name="warm", bufs=1) as warm,
            tc.tile_pool(name="psum", bufs=6, space="PSUM") as pp,
            tc.tile_pool(name="psumw", bufs=1, space="PSUM") as ppw,
        ):
            # PE warmup: the HAM clock gate only reaches 2.4 GHz after ~3.4us
            # of sustained PE activity. Burn the initial DMA wait on dummy
            # matmuls so the real ones start at full clock.
            wu = warm.tile([P, 256], mybir.dt.bfloat16, name="wu")
            nc.gpsimd.memset(wu[:], 0.0)
            wups = ppw.tile([P, 256], mybir.dt.float32, name="wups")
            for _ in range(WARMUP_MMS):
                nc.tensor.matmul(wups[:], wu[:, :P], wu[:], start=True, stop=True)

            # Resident xT tiles: [k-partition, k-tile, token] per m-tile.
            # w streams as [128, KG, 512] sub-tiles (512 KB) so matmuls wait
            # on small DMAs; 16 pool slots hold the live chunk plus a fully
            # prefetched next chunk. All loads share the sync engine's HWDGE
            # queue: the single FIFO keeps the ramp's arrival order exactly
            # the consumption order (a second engine's stream interleaves on
            # the DMA rings and delays the pieces the PE is waiting on).
            w_tiles = [None] * NT

            def w_sub(nt, g):
                wt = wpool.tile(
                    [P, KG, NF], mybir.dt.bfloat16, name=f"w_{nt}_{g}", tag="w"
                )
                nc.sync.dma_start(out=wt[:], in_=ws_d[nt, :, g * KG:(g + 1) * KG, :])
                return wt

            def load_w(nt):
                w_tiles[nt] = [w_sub(nt, g) for g in range(NSUB)]

            def x_tile(mt):
                xt = xpool.tile([P, KT, P], mybir.dt.bfloat16, name=f"xs_t{mt}")
                nc.sync.dma_start(out=xt[:], in_=xs_d[mt])
                return xt

            # Ramp: first two m-tiles in k-halves (512 KB), interleaved with
            # the first w chunk's pieces in exactly the order the
            # pair-interleaved matmuls below consume them.
            KH = KT // 2
            xs_sub = {0: [], 1: []}

            def x_half(mt, h):
                xh = xpool.tile([P, KH, P], mybir.dt.bfloat16, name=f"xs_t{mt}_{h}")
                nc.sync.dma_start(
                    out=xh[:], in_=xs_d[mt, :, h * KH:(h + 1) * KH, :]
                )
                xs_sub[mt].append(xh)

            x_half(0, 0)
            w0 = [w_sub(0, 0)]
            x_half(1, 0)
            w0 += [w_sub(0, g) for g in range(1, NSUB // 2)]
            x_half(0, 1)
            x_half(1, 1)
            w0 += [w_sub(0, g) for g in range(NSUB // 2, NSUB)]
            w_tiles[0] = w0

            xs_t = [None, None]
            for mt in range(2, MT):
                xs_t.append(x_tile(mt))

            def x_slice(mt, ko):
                if mt < 2:
                    return xs_sub[mt][ko // KH][:, ko % KH, :]
                return xs_t[mt][:, ko, :]

            def w_slice(nt, ko):
                return w_tiles[nt][ko // KG][:, ko % KG, :]

            def finish_tile(nt, mt, ps):
                ev = epool.tile([P, NF], mybir.dt.float32, name="ev", tag="ev")
                nc.vector.tensor_copy(ev[:], ps[:])
                nc.sync.dma_start(
                    out=out_d[mt * P:(mt + 1) * P, nt * NF:(nt + 1) * NF],
                    in_=ev[:],
                )

            for nt in range(NT):
                if w_tiles[nt] is None:
                    load_w(nt)
                if nt == 0:
                    # Ramp: the first w chunk is still streaming in, and the
                    # PE eats one (m-tile, w-sub) block faster than its DMA.
                    # Interleave m-tile pairs (two open PSUM groups) so each
                    # w sub-tile feeds 2x the PE work and the DMA keeps up
                    # from the very first matmul.
                    for mp in range(0, 4, 2):
                        ps_a = pp.tile([P, NF], mybir.dt.float32, name="ps", tag="ps")
                        ps_b = pp.tile([P, NF], mybir.dt.float32, name="ps2", tag="ps")
                        for g in range(NSUB):
                            for mt, ps in ((mp, ps_a), (mp + 1, ps_b)):
                                for kk in range(KG):
                                    ko = g * KG + kk
                                    nc.tensor.matmul(
                                        ps[:],
                                        x_slice(mt, ko),
                                        w_slice(nt, ko),
                                        start=(ko == 0),
                                        stop=(ko == KT - 1),
                                    )
                        finish_tile(nt, mp, ps_a)
                        finish_tile(nt, mp + 1, ps_b)
                    mts = range(4, MT)
                else:
                    mts = range(MT)
                for mt in mts:
                    ps = pp.tile([P, NF], mybir.dt.float32, name="ps", tag="ps")
                    for ko in range(KT):
                        nc.tensor.matmul(
                            ps[:],
                            x_slice(mt, ko),
                            w_slice(nt, ko),
                            start=(ko == 0),
                            stop=(ko == KT - 1),
                        )
                    finish_tile(nt, mt, ps)

    nc.compile()
    return nc


def _prep_inputs(x, kern, scale):
    """Host-side: fold scale into ternary bf16 weights; cast+tile x per core."""
    s = float(np.asarray(scale))
    # w[k, f] = +-scale, exact in bf16 when scale is a power of two.
    w = np.where(np.asarray(kern), np.float32(s), np.float32(-s)).astype(_BF16)
    # ws[nt, kp, ko, n] = w[ko*128 + kp, nt*512 + n]
    ws = np.ascontiguousarray(
        w.reshape(KT, P, NT, NF).transpose(2, 1, 0, 3)
    )

    xf = np.asarray(x).reshape(TOKENS, IN_DIM).astype(_BF16)
    in_maps = []
    for c in range(N_CORES):
        xc = xf[c * TOK_PER_CORE:(c + 1) * TOK_PER_CORE]
        # xs[mt, kp, ko, mi] = xc[mt*128 + mi, ko*128 + kp]
        xs = np.ascontiguousarray(
            xc.reshape(MT, P, KT, P).transpose(0, 3, 2, 1)
        )
        in_maps.append({"xs": xs, "ws": ws})
    return in_maps


def _run(inputs, trace=False, tmpdir=None):
    from concourse.bass_utils import run_bass_kernel_spmd

    if "nc" not in _cache:
        _cache["nc"] = _build_program()
    nc = _cache["nc"]

    in_maps = _prep_inputs(inputs["x"], inputs["kernel"], inputs["scale"])
    res = run_bass_kernel_spmd(
        nc, in_maps, core_ids=list(range(N_CORES)), trace=trace, tmpdir=tmpdir
    )
    out = np.concatenate(
        [res.results[c]["out"][None] for c in range(N_CORES)], axis=0
    ).reshape(BATCH, SEQ, FEATURES)
    return np.ascontiguousarray(out.astype(np.float32, copy=False)), res


def kernel(**inputs):
    out, _ = _run(inputs, trace=False)
    return out


# revision 26
# speedup vs baseline: 1.0084x; 1.0001x over previous
"""Trainium2 Bass kernel for nn_Dense_56779467653682.

Computes out = scale * x @ (2*kernel - 1) where x:[8,2048,4096] f32,
kernel:[4096,4096] bool, scale scalar f32 (= 1/64).

Strategy: data-parallel over the 16384 tokens across 8 NeuronCores
(2048 tokens/core). The ternary weight (+-scale, exact in bf16 since
scale is a power of two) is folded on the host into a bf16 weight
matrix, and x is cast to bf16 and pre-transposed/tiled on the host so
the device kernel is a pure dense matmul:

    per core: out[2048, 4096] f32 = x_bf16[2048, 4096] @ w_bf16[4096, 4096]

Device tiling (per core):
  - contraction K=4096 -> 32 k-tiles of 128 (partition dim)
  - tokens M=2048 -> 16 m-tiles of 128 (PSUM partition dim, lhsT free dim)
  - features N=4096 -> 8 n-chunks of 512 (PSUM free dim = one bank)
  All 16 xT m-tiles stay resident in SBUF (128 KB/partition); w streams
  once in 4 MB n-chunks (double buffered); each output tile accumulates
  32 back-to-back matmuls in one PSUM bank, is copied to SBUF on the
  DVE, and DMA'd out.
"""

import numpy as np
import ml_dtypes

BATCH, SEQ, IN_DIM, FEATURES = 8, 2048, 4096, 4096
N_CORES = 8
TOKENS = BATCH * SEQ
TOK_PER_CORE = TOKENS // N_CORES  # 2048
P = 128                           # partitions / tile edge
KT = IN_DIM // P                  # 32 k-tiles
MT = TOK_PER_CORE // P            # 16 m-tiles
NF = 512                          # features per n-chunk (one PSUM bank of f32)
NT = FEATURES // NF               # 8 n-chunks

_BF16 = ml_dtypes.bfloat16

_cache = {}


def _build_program():
    """Build + compile the per-core Bass/Tile program (SPMD, same on all cores)."""
    import concourse.bacc as bacc
    import concourse.mybir as mybir
    from concourse.tile import TileContext

    nc = bacc.Bacc("TRN2", target_bir_lowering=False, debug=False)

    xs_d = nc.dram_tensor("xs", [MT, P, KT, P], mybir.dt.bfloat16, kind="ExternalInput")
    ws_d = nc.dram_tensor("ws", [NT, P, KT, NF], mybir.dt.bfloat16, kind="ExternalInput")
    out_d = nc.dram_tensor("out", [TOK_PER_CORE, FEATURES], mybir.dt.float32, kind="ExternalOutput")

    KG = 4                 # k-tiles per w sub-tile (fine-grained RAW deps)
    NSUB = KT // KG        # 8 sub-tiles per n-chunk
    WARMUP_MMS = 20        # dummy matmuls to lift HAM to K=8/8 during input DMA

    with TileContext(nc) as tc:
        with (
            tc.tile_pool(name="xpool", bufs=1) as xpool,
            tc.tile_pool(name="wpool", bufs=2 * NSUB) as wpool,
            tc.tile_pool(name="epool", bufs=4) as epool,
            tc.tile_pool(name="warm", bufs=1) as warm,
            tc.tile_pool(name="psum", bufs=6, space="PSUM") as pp,
            tc.tile_pool(name="psumw", bufs=1, space="PSUM") as ppw,
        ):
            # PE warmup: the HAM clock gate only reaches 2.4 GHz after ~3.4us
            # of sustained PE activity. Burn the initial DMA wait on dummy
            # matmuls so the real ones start at full clock.
            wu = warm.tile([P, 256], mybir.dt.bfloat16, name="wu")
            nc.gpsimd.memset(wu[:], 0.0)
            wups = ppw.tile([P, 256], mybir.dt.float32, name="wups")
            for _ in range(WARMUP_MMS):
                nc.tensor.matmul(wups[:], wu[:, :P], wu[:], start=True, stop=True)

            # Resident xT tiles: [k-partition, k-tile, token] per m-tile.
            # w streams as [128, KG, 512] sub-tiles (512 KB) so matmuls wait
            # on small DMAs; 16 pool slots hold the live chunk plus a fully
            # prefetched next chunk. All loads share the sync engine's HWDGE
            # queue: the single FIFO keeps the ramp's arrival order exactly
            # the consumption order (a second engine's stream interleaves on
            # the DMA rings and delays the pieces the PE is waiting on).
            w_tiles = [None] * NT

            def w_sub(nt, g):
                wt = wpool.tile(
                    [P, KG, NF], mybir.dt.bfloat16, name=f"w_{nt}_{g}", tag="w"
                )
                nc.sync.dma_start(out=wt[:], in_=ws_d[nt, :, g * KG:(g + 1) * KG, :])
                return wt

            def load_w(nt):
                w_tiles[nt] = [w_sub(nt, g) for g in range(NSUB)]

            def x_tile(mt):
                xt = xpool.tile([P, KT, P], mybir.dt.bfloat16, name=f"xs_t{mt}")
                nc.sync.dma_start(out=xt[:], in_=xs_d[mt])
                return xt

            # Ramp: first two m-tiles in k-halves (512 KB), interleaved with
            # the first w chunk's pieces in exactly the order the
            # pair-interleaved matmuls below consume them.
            KH = KT // 2
            xs_sub = {0: [], 1: []}

            def x_half(mt, h):
                xh = xpool.tile([P, KH, P], mybir.dt.bfloat16, name=f"xs_t{mt}_{h}")
                nc.sync.dma_start(
                    out=xh[:], in_=xs_d[mt, :, h * KH:(h + 1) * KH, :]
                )
                xs_sub[mt].append(xh)

            # Arrival order tuned against piece-level consumption: mt0 runs
            # solo through w pieces 0-1 (below), so w0[1] is needed before
            # x1's first half.
            x_half(0, 0)
            w0 = [w_sub(0, 0), w_sub(0, 1)]
            x_half(1, 0)
            w0 += [w_sub(0, g) for g in range(2, NSUB // 2)]
            x_half(0, 1)
            x_half(1, 1)
            w0 += [w_sub(0, g) for g in range(NSUB // 2, NSUB)]
            w_tiles[0] = w0

            xs_t = [None, None]
            for mt in range(2, MT):
                xs_t.append(x_tile(mt))

            def x_slice(mt, ko):
                if mt < 2:
                    return xs_sub[mt][ko // KH][:, ko % KH, :]
                return xs_t[mt][:, ko, :]

            def w_slice(nt, ko):
                return w_tiles[nt][ko // KG][:, ko % KG, :]

            def finish_tile(nt, mt, ps):
                ev = epool.tile([P, NF], mybir.dt.float32, name="ev", tag="ev")
                nc.vector.tensor_copy(ev[:], ps[:])
                nc.sync.dma_start(
                    out=out_d[mt * P:(mt + 1) * P, nt * NF:(nt + 1) * NF],
                    in_=ev[:],
                )

            for nt in range(NT):
                if w_tiles[nt] is None:
                    load_w(nt)
                if nt == 0:
                    # Ramp: the first w chunk is still streaming in, and the
                    # PE eats one (m-tile, w-sub) block faster than its DMA.
                    # Interleave m-tile pairs (two open PSUM groups) so each
                    # w sub-tile feeds 2x the PE work and the DMA keeps up
                    # from the very first matmul. mt0 runs solo through the
                    # first two pieces (x1's first half lands after w0[1]),
                    # then mt1 catches up and the pair interleaves.
                    for mp in range(0, 4, 2):
                        ps_a = pp.tile([P, NF], mybir.dt.float32, name="ps", tag="ps")
                        ps_b = pp.tile([P, NF], mybir.dt.float32, name="ps2", tag="ps")
                        if mp == 0:
                            for ko in range(2 * KG):
                                nc.tensor.matmul(
                                    ps_a[:], x_slice(0, ko), w_slice(0, ko),
                                    start=(ko == 0), stop=False,
                                )
                            for ko in range(2 * KG):
                                nc.tensor.matmul(
                                    ps_b[:], x_slice(1, ko), w_slice(0, ko),
                                    start=(ko == 0), stop=False,
                                )
                            g0 = 2
                        else:
                            g0 = 0
                        for g in range(g0, NSUB):
                            for mt, ps in ((mp, ps_a), (mp + 1, ps_b)):
                                for kk in range(KG):
                                    ko = g * KG + kk
                                    nc.tensor.matmul(
                                        ps[:],
                                        x_slice(mt, ko),
                                        w_slice(nt, ko),
                                        start=(ko == 0),
                                        stop=(ko == KT - 1),
                                    )
                        finish_tile(nt, mp, ps_a)
                        finish_tile(nt, mp + 1, ps_b)
                    mts = range(4, MT)
                else:
                    mts = range(MT)
                for mt in mts:
                    ps = pp.tile([P, NF], mybir.dt.float32, name="ps", tag="ps")
                    for ko in range(KT):
                        nc.tensor.matmul(
                            ps[:],
                            x_slice(mt, ko),
                            w_slice(nt, ko),
                            start=(ko == 0),
                            stop=(ko == KT - 1),
                        )
                    finish_tile(nt, mt, ps)

    nc.compile()
    return nc


def _prep_inputs(x, kern, scale):
    """Host-side: fold scale into ternary bf16 weights; cast+tile x per core."""
    s = float(np.asarray(scale))
    # w[k, f] = +-scale, exact in bf16 when scale is a power of two.
    w = np.where(np.asarray(kern), np.float32(s), np.float32(-s)).astype(_BF16)
    # ws[nt, kp, ko, n] = w[ko*128 + kp, nt*512 + n]
    ws = np.ascontiguousarray(
        w.reshape(KT, P, NT, NF).transpose(2, 1, 0, 3)
    )

    xf = np.asarray(x).reshape(TOKENS, IN_DIM).astype(_BF16)
    in_maps = []
    for c in range(N_CORES):
        xc = xf[c * TOK_PER_CORE:(c + 1) * TOK_PER_CORE]
        # xs[mt, kp, ko, mi] = xc[mt*128 + mi, ko*128 + kp]
        xs = np.ascontiguousarray(
            xc.reshape(MT, P, KT, P).transpose(0, 3, 2, 1)
        )
        in_maps.append({"xs": xs, "ws": ws})
    return in_maps


def _run(inputs, trace=False, tmpdir=None):
    from concourse.bass_utils import run_bass_kernel_spmd

    if "nc" not in _cache:
        _cache["nc"] = _build_program()
    nc = _cache["nc"]

    in_maps = _prep_inputs(inputs["x"], inputs["kernel"], inputs["scale"])
    res = run_bass_kernel_spmd(
        nc, in_maps, core_ids=list(range(N_CORES)), trace=trace, tmpdir=tmpdir
    )
    out = np.concatenate(
        [res.results[c]["out"][None] for c in range(N_CORES)], axis=0
    ).reshape(BATCH, SEQ, FEATURES)
    return np.ascontiguousarray(out.astype(np.float32, copy=False)), res


def kernel(**inputs):
    out, _ = _run(inputs, trace=False)
    return out


# revision 28
# speedup vs baseline: 1.0089x; 1.0005x over previous
"""Trainium2 Bass kernel for nn_Dense_56779467653682.

Computes out = scale * x @ (2*kernel - 1) where x:[8,2048,4096] f32,
kernel:[4096,4096] bool, scale scalar f32 (= 1/64).

Strategy: data-parallel over the 16384 tokens across 8 NeuronCores
(2048 tokens/core). The ternary weight (+-scale, exact in bf16 since
scale is a power of two) is folded on the host into a bf16 weight
matrix, and x is cast to bf16 and pre-transposed/tiled on the host so
the device kernel is a pure dense matmul:

    per core: out[2048, 4096] f32 = x_bf16[2048, 4096] @ w_bf16[4096, 4096]

Device tiling (per core):
  - contraction K=4096 -> 32 k-tiles of 128 (partition dim)
  - tokens M=2048 -> 16 m-tiles of 128 (PSUM partition dim, lhsT free dim)
  - features N=4096 -> 8 n-chunks of 512 (PSUM free dim = one bank)
  All 16 xT m-tiles stay resident in SBUF (128 KB/partition); w streams
  once in 4 MB n-chunks (double buffered); each output tile accumulates
  32 back-to-back matmuls in one PSUM bank, is copied to SBUF on the
  DVE, and DMA'd out.
"""

import numpy as np
import ml_dtypes

BATCH, SEQ, IN_DIM, FEATURES = 8, 2048, 4096, 4096
N_CORES = 8
TOKENS = BATCH * SEQ
TOK_PER_CORE = TOKENS // N_CORES  # 2048
P = 128                           # partitions / tile edge
KT = IN_DIM // P                  # 32 k-tiles
MT = TOK_PER_CORE // P            # 16 m-tiles
NF = 512                          # features per n-chunk (one PSUM bank of f32)
NT = FEATURES // NF               # 8 n-chunks

_BF16 = ml_dtypes.bfloat16

_cache = {}


def _build_program():
    """Build + compile the per-core Bass/Tile program (SPMD, same on all cores)."""
    import concourse.bacc as bacc
    import concourse.mybir as mybir
    from concourse.tile import TileContext

    nc = bacc.Bacc("TRN2", target_bir_lowering=False, debug=False)

    xs_d = nc.dram_tensor("xs", [MT, P, KT, P], mybir.dt.bfloat16, kind="ExternalInput")
    ws_d = nc.dram_tensor("ws", [NT, P, KT, NF], mybir.dt.bfloat16, kind="ExternalInput")
    out_d = nc.dram_tensor("out", [TOK_PER_CORE, FEATURES], mybir.dt.float32, kind="ExternalOutput")

    KG = 4                 # k-tiles per w sub-tile (fine-grained RAW deps)
    NSUB = KT // KG        # 8 sub-tiles per n-chunk
    WARMUP_MMS = 20        # dummy matmuls to lift HAM to K=8/8 during input DMA

    with TileContext(nc) as tc:
        with (
            tc.tile_pool(name="xpool", bufs=1) as xpool,
            tc.tile_pool(name="wpool", bufs=2 * NSUB) as wpool,
            tc.tile_pool(name="epool", bufs=4) as epool,
            tc.tile_pool(name="warm", bufs=1) as warm,
            tc.tile_pool(name="psum", bufs=6, space="PSUM") as pp,
            tc.tile_pool(name="psumw", bufs=1, space="PSUM") as ppw,
        ):
            # PE warmup: the HAM clock gate only reaches 2.4 GHz after ~3.4us
            # of sustained PE activity. Burn the initial DMA wait on dummy
            # matmuls so the real ones start at full clock.
            wu = warm.tile([P, 256], mybir.dt.bfloat16, name="wu")
            nc.gpsimd.memset(wu[:], 0.0)
            wups = ppw.tile([P, 256], mybir.dt.float32, name="wups")
            for _ in range(WARMUP_MMS):
                nc.tensor.matmul(wups[:], wu[:, :P], wu[:], start=True, stop=True)

            # Resident xT tiles: [k-partition, k-tile, token] per m-tile.
            # w streams as [128, KG, 512] sub-tiles (512 KB) so matmuls wait
            # on small DMAs; 16 pool slots hold the live chunk plus a fully
            # prefetched next chunk. All loads share the sync engine's HWDGE
            # queue: the single FIFO keeps the ramp's arrival order exactly
            # the consumption order (a second engine's stream interleaves on
            # the DMA rings and delays the pieces the PE is waiting on).
            w_tiles = [None] * NT

            def w_sub(nt, g):
                wt = wpool.tile(
                    [P, KG, NF], mybir.dt.bfloat16, name=f"w_{nt}_{g}", tag="w"
                )
                nc.sync.dma_start(out=wt[:], in_=ws_d[nt, :, g * KG:(g + 1) * KG, :])
                return wt

            def load_w(nt):
                w_tiles[nt] = [w_sub(nt, g) for g in range(NSUB)]

            def x_tile(mt):
                xt = xpool.tile([P, KT, P], mybir.dt.bfloat16, name=f"xs_t{mt}")
                nc.sync.dma_start(out=xt[:], in_=xs_d[mt])
                return xt

            # Ramp: first two m-tiles in k-halves (512 KB), interleaved with
            # the first w chunk's pieces in exactly the order the
            # pair-interleaved matmuls below consume them.
            KH = KT // 2
            xs_sub = {0: [], 1: []}

            def x_half(mt, h):
                xh = xpool.tile([P, KH, P], mybir.dt.bfloat16, name=f"xs_t{mt}_{h}")
                nc.sync.dma_start(
                    out=xh[:], in_=xs_d[mt, :, h * KH:(h + 1) * KH, :]
                )
                xs_sub[mt].append(xh)

            # Arrival order tuned against piece-level consumption: mt0 runs
            # solo through w pieces 0-1 (below), so w0[1] is needed before
            # x1's first half.
            x_half(0, 0)
            w0 = [w_sub(0, 0), w_sub(0, 1)]
            x_half(1, 0)
            w0 += [w_sub(0, g) for g in range(2, NSUB // 2)]
            x_half(0, 1)
            x_half(1, 1)
            w0 += [w_sub(0, g) for g in range(NSUB // 2, NSUB)]
            w_tiles[0] = w0

            xs_t = [None, None]
            for mt in range(2, MT):
                xs_t.append(x_tile(mt))

            def x_slice(mt, ko):
                if mt < 2:
                    return xs_sub[mt][ko // KH][:, ko % KH, :]
                return xs_t[mt][:, ko, :]

            def w_slice(nt, ko):
                return w_tiles[nt][ko // KG][:, ko % KG, :]

            def finish_tile(nt, mt, ps):
                ev = epool.tile([P, NF], mybir.dt.float32, name="ev", tag="ev")
                nc.vector.tensor_copy(ev[:], ps[:])
                nc.sync.dma_start(
                    out=out_d[mt * P:(mt + 1) * P, nt * NF:(nt + 1) * NF],
                    in_=ev[:],
                )

            for nt in range(NT):
                if w_tiles[nt] is None:
                    load_w(nt)
                if nt == 0:
                    # Ramp: the first w chunk is still streaming in, and the
                    # PE eats one (m-tile, w-sub) block faster than its DMA.
                    # Interleave m-tile pairs (two open PSUM groups) so each
                    # w sub-tile feeds 2x the PE work and the DMA keeps up
                    # from the very first matmul. mt0 runs solo through the
                    # first two pieces (x1's first half lands after w0[1]),
                    # then mt1 catches up and the pair interleaves.
                    for mp in range(0, 4, 2):
                        ps_a = pp.tile([P, NF], mybir.dt.float32, name="ps", tag="ps")
                        ps_b = pp.tile([P, NF], mybir.dt.float32, name="ps2", tag="ps")
                        if mp == 0:
                            for ko in range(2 * KG):
                                nc.tensor.matmul(
                                    ps_a[:], x_slice(0, ko), w_slice(0, ko),
                                    start=(ko == 0), stop=False,
                                )
                            for ko in range(2 * KG):
                                nc.tensor.matmul(
                                    ps_b[:], x_slice(1, ko), w_slice(0, ko),
                                    start=(ko == 0), stop=False,
                                )
                            g0 = 2
                        else:
                            g0 = 0
                        for g in range(g0, NSUB):
                            for mt, ps in ((mp, ps_a), (mp + 1, ps_b)):
                                for kk in range(KG):
                                    ko = g * KG + kk
                                    nc.tensor.matmul(
                                        ps[:],
                                        x_slice(mt, ko),
                                        w_slice(nt, ko),
                                        start=(ko == 0),
                                        stop=(ko == KT - 1),
                                    )
                        finish_tile(nt, mp, ps_a)
                        finish_tile(nt, mp + 1, ps_b)
                    mts = range(4, MT)
                else:
                    mts = range(MT)
                for mt in mts:
                    ps = pp.tile([P, NF], mybir.dt.float32, name="ps", tag="ps")
                    for ko in range(KT):
                        nc.tensor.matmul(
                            ps[:],
                            x_slice(mt, ko),
                            w_slice(nt, ko),
                            start=(ko == 0),
                            stop=(ko == KT - 1),
                        )
                    finish_tile(nt, mt, ps)

    nc.compile()
    return nc


def _prep_inputs(x, kern, scale):
    """Host-side: fold scale into ternary bf16 weights; cast+tile x per core."""
    s = float(np.asarray(scale))
    # w[k, f] = +-scale, exact in bf16 when scale is a power of two.
    w = np.where(np.asarray(kern), np.float32(s), np.float32(-s)).astype(_BF16)
    # ws[nt, kp, ko, n] = w[ko*128 + kp, nt*512 + n]
    ws = np.ascontiguousarray(
        w.reshape(KT, P, NT, NF).transpose(2, 1, 0, 3)
    )

    xf = np.asarray(x).reshape(TOKENS, IN_DIM).astype(_BF16)
    in_maps = []
    for c in range(N_CORES):
        xc = xf[c * TOK_PER_CORE:(c + 1) * TOK_PER_CORE]
        # xs[mt, kp, ko, mi] = xc[mt*128 + mi, ko*128 + kp]
        xs = np.ascontiguousarray(
            xc.reshape(MT, P, KT, P).transpose(0, 3, 2, 1)
        )
        in_maps.append({"xs": xs, "ws": ws})
    return in_maps


def _run(inputs, trace=False, tmpdir=None):
    from concourse.bass_utils import run_bass_kernel_spmd

    if "nc" not in _cache:
        _cache["nc"] = _build_program()
    nc = _cache["nc"]

    in_maps = _prep_inputs(inputs["x"], inputs["kernel"], inputs["scale"])
    res = run_bass_kernel_spmd(
        nc, in_maps, core_ids=list(range(N_CORES)), trace=trace, tmpdir=tmpdir
    )
    out = np.concatenate(
        [res.results[c]["out"][None] for c in range(N_CORES)], axis=0
    ).reshape(BATCH, SEQ, FEATURES)
    return np.ascontiguousarray(out.astype(np.float32, copy=False)), res


def kernel(**inputs):
    out, _ = _run(inputs, trace=False)
    return out
